# revision 73
# baseline (speedup 1.0000x reference)
"""LSTM cell kernel for Trainium2, SPMD over 8 NeuronCores.

Problem: nn_LstmCell — B=8192, D_IN=D_H=2048.
    g = x @ Wx.T + bx + h @ Wh.T + bh          # [B, 3H]
    gi, gm, go = split(g, 3)
    c_new = sigmoid(gm)*c + sigmoid(gi)*tanh(gm)
    h_new = sigmoid(go)*tanh(c_new)

Strategy:
  - Data-parallel over batch: each core owns 1024 rows of x/h/c.
  - Fused GEMM computed transposed (gates on PSUM partitions, batch on free
    dim) so the per-gate bias folds into the ScalarE activation and
    sigmoid/tanh read PSUM directly.
  - fp8 e4m3 matmul inputs in DoubleRow perf mode (2 k-tiles contracted per
    instruction at 0.5 cycles/row): 4x PE throughput vs bf16.
  - Accuracy: inputs are pre-scaled (A*16, W*512) into e4m3's sweet spot and
    the 1/8192 un-scale folds into the activation's scale operand. The m-gate
    (used for BOTH tanh candidate and forget sigmoid — the error-critical
    path) gets a residual-compensated product:
        gm = A8@Wm8 + A8@WmLO + ALO[:, :3K/4]@Wm8
    where WmLO/ALO are e4m3 quantization residuals at the same scale, all
    accumulated into one PSUM bank. Device-measured rel_err 1.705e-2 (h),
    1.074e-2 (c) vs the 2e-2 gate; bit-identical to the numpy interp.
  - Software pipelining: each tile's ALO pass + m epilogue run under the
    NEXT tile's matmuls (m PSUM banks double-buffered) so the 3.2MB ALO
    tensor stays out of the congested early DMA window; epilogue stages
    fire per-gate as their PSUM chains close, leaving only
    s_o -> h_new -> DMA on the kernel tail.
  - Weights streamed from HBM (one pass), activations resident in SBUF,
    bf16 epilogue/outputs (DVE 2x mode, half the output DMA bytes).

Host-side: layout transforms + fp8 casts (not counted in HW exec time).
Timeline-sim HW exec: ~279us vs the 685us bf16 baseline (2.46x).
"""

import os

import numpy as np
import ml_dtypes

N_CORES = 8
B = 8192
DH = 2048            # latent dim (= D_IN = D_H)
H3 = 3 * DH          # 6144 gate rows
K = 2 * DH           # 4096 contraction dim
BLOC = B // N_CORES  # 1024 batch rows per core
P = 128
KT = K // P          # 32 k-tiles
K2 = KT // 2         # 16 double-row k-pairs
MT = H3 // P         # 48 gate-row tiles
DTL = DH // P        # 16 d-tiles per gate
NF = 512             # matmul free dim (one PSUM bank of fp32)
NH = BLOC // NF      # 2 batch halves

SA = 16.0            # activation pre-scale before e4m3 cast
SW = 512.0           # weight pre-scale before e4m3 cast
INV = 1.0 / (SA * SW)
AKT = 24             # k-tiles covered by the A-residual (ALO) pass: 3/4 of K
AK2 = AKT // 2       # 12 double-row k-pairs in the ALO pass

_E4 = ml_dtypes.float8_e4m3

_CACHE = {}
LAST_RESULT = None  # BassKernelResults from the most recent run (for test.py)


def _split_multiwaits(nc):
    """This container's walrus build rejects >1 sync-wait on an engine
    instruction ("Too many sync wait commands"). Split extra waits into
    standalone EventSemaphore instructions on the same engine immediately
    before the instruction (same stall semantics: engines are in-order)."""
    import concourse.mybir as mybir

    f = nc.m.functions[0]
    for blk in f.blocks:
        new_insts = []
        for inst in blk.instructions:
            si = getattr(inst, "sync_info", None)
            ow = list(si.on_wait) if (si is not None and si.on_wait) else []
            if len(ow) > 1:
                for w in ow[:-1]:
                    new_insts.append(
                        mybir.InstEventSemaphore(
                            name=nc.get_next_instruction_name(),
                            engine=inst.engine,
                            ins=[],
                            outs=[],
                            sync_info=mybir.SyncInfo(on_wait=[w], on_update=[]),
                        )
                    )
                inst.sync_info = mybir.SyncInfo(
                    on_wait=[ow[-1]], on_update=list(si.on_update)
                )
            new_insts.append(inst)
        blk.instructions[:] = new_insts


def _build_bass(dtl=DTL):
    import concourse.bass as bass
    import concourse.mybir as mybir
    import concourse.tile as tile

    f32 = mybir.dt.float32
    bf16 = mybir.dt.bfloat16
    fp8 = mybir.dt.float8e4
    AF = mybir.ActivationFunctionType
    DR = mybir.MatmulPerfMode.DoubleRow

    nc = bass.Bass("TRN2", name="lstm_cell")

    W8 = nc.dram_tensor("W8", [MT, P, KT, P], fp8, kind="ExternalInput")
    WLO = nc.dram_tensor("WLO", [DTL, P, KT, P], fp8, kind="ExternalInput")
    A8 = nc.dram_tensor("A8", [P, KT, BLOC], fp8, kind="ExternalInput")
    ALO = nc.dram_tensor("ALO", [P, AKT, BLOC], fp8, kind="ExternalInput")
    CT = nc.dram_tensor("CT", [DH, BLOC], bf16, kind="ExternalInput")
    BIAS = nc.dram_tensor("BIAS", [P, MT], f32, kind="ExternalInput")
    HT = nc.dram_tensor("HT", [DH, BLOC], bf16, kind="ExternalOutput")
    CNT = nc.dram_tensor("CNT", [DH, BLOC], bf16, kind="ExternalOutput")

    with tile.TileContext(nc) as tc:
        with (
            tc.tile_pool(name="const", bufs=1) as const_pool,
            tc.tile_pool(name="wpool", bufs=2) as wpool,
            tc.tile_pool(name="wpool_m", bufs=3) as wpool_m,
            tc.tile_pool(name="cpool", bufs=2) as cpool,
            tc.tile_pool(name="epool", bufs=3) as epool,
            tc.tile_pool(name="psum_io", bufs=1, space="PSUM") as psum_io,
            tc.tile_pool(name="psum_m", bufs=2, space="PSUM") as psum_m,
        ):
            # Activations resident in SBUF, loaded as graded HWDGE slices
            # (SWDGE chunk loads would serialize ~1us/chunk on Pool): the
            # first d-tile's k-major matmuls stream right behind them.
            a_sb = const_pool.tile([P, KT, BLOC], fp8, name="a_sb")
            a_slices = [2, 2, 4, 4, 4, 8, 4, 4]  # graded: fast first tiles
            kg = 0
            for sl in a_slices:
                nc.scalar.dma_start(
                    a_sb[:, kg : kg + sl, :],
                    A8[:, kg : kg + sl, :],
                )
                kg += sl
            # ALO lives in SBUF too, but its loads are issued from d1's
            # block, behind the A8 slices on the ACT queue (first reader is
            # the deferred d0 ALO pass at the end of d1; issuing at t=0
            # would preempt A8's tail slices on the shared DMA engines).
            AQ = KT // 4
            alo_sb = const_pool.tile([P, AKT, BLOC], fp8, name="alo_sb")
            bias_sb = const_pool.tile([P, MT], f32, name="bias_sb")

            # pipeline state for the previous tile whose m chain is still
            # open: (d, psums, m_strip, s_i_tiles, s_o_tiles)
            prev = None

            def m_mid_epilogue(d_, psums_, s_i_tiles_, c_tiles_):
                """t_m/s_m + cell update through tanh(c_new) for d-tile d_
                (whose m chain just closed). Returns per-nh (t_c, c_new)."""
                b_m = bias_sb[:, DTL + d_ : DTL + d_ + 1]
                mid = []
                for nh in range(NH):
                    t_m = epool.tile([P, NF], f32, name="t_m", tag="t_m")
                    s_m = epool.tile([P, NF], f32, name="s_m", tag="s_m")
                    part = epool.tile([P, NF], f32, name="part", tag="part")
                    fc = epool.tile([P, NF], f32, name="fc", tag="fc")
                    c_new = epool.tile([P, NF], f32, name="c_new", tag="c_new")
                    t_c = epool.tile([P, NF], f32, name="t_c", tag="t_c")

                    nc.scalar.activation(
                        t_m[:], psums_[("m", nh)][:], AF.Tanh, bias=b_m, scale=INV
                    )
                    nc.scalar.activation(
                        s_m[:], psums_[("m", nh)][:], AF.Sigmoid, bias=b_m, scale=INV
                    )
                    nc.vector.tensor_mul(part[:], s_i_tiles_[nh][:], t_m[:])
                    nc.vector.tensor_mul(fc[:], s_m[:], c_tiles_[nh][:])
                    nc.vector.tensor_add(c_new[:], fc[:], part[:])
                    nc.scalar.activation(t_c[:], c_new[:], AF.Tanh)
                    nc.scalar.dma_start(
                        CNT[d_ * P : (d_ + 1) * P, nh * NF : (nh + 1) * NF],
                        c_new[:],
                    )
                    mid.append(t_c)
                return mid

            def s_o_epilogue(d_, psums_):
                """sigmoid(go): drains the o PSUM banks."""
                b_o = bias_sb[:, 2 * DTL + d_ : 2 * DTL + d_ + 1]
                out = []
                for nh in range(NH):
                    s_o = epool.tile([P, NF], f32, name="s_o", tag="s_o")
                    nc.scalar.activation(
                        s_o[:], psums_[("o", nh)][:], AF.Sigmoid, bias=b_o, scale=INV
                    )
                    out.append(s_o)
                return out

            def h_epilogue(d_, s_o_tiles_, mid_, out_eng):
                """h = sigmoid(go) * tanh(c_new) + h output for d-tile d_."""
                for nh in range(NH):
                    h_new = epool.tile([P, NF], f32, name="h_new", tag="h_new")
                    nc.vector.tensor_mul(h_new[:], s_o_tiles_[nh][:], mid_[nh][:])
                    out_eng.dma_start(
                        HT[d_ * P : (d_ + 1) * P, nh * NF : (nh + 1) * NF],
                        h_new[:],
                    )

            for d in range(dtl):
                # Stream this d-tile's weight strips (0.5 MB each, fp8):
                # 3 gate strips + the m-gate residual strip. d0's strips go
                # in k-halves split across the SP and Pool queues (a single
                # queue only issues one HWDGE DMA per ~1.2us), interleaved so
                # every strip's first half lands before any second half: the
                # k-major matmuls start sooner.
                HK = KT // 2
                strips = {}
                for gi_, g in enumerate("imo"):
                    # the m strip is read across TWO blocks (its own m_hi
                    # pass + the deferred ALO pass in the next block), so it
                    # triple-buffers: with bufs=2 the load two tiles later
                    # would stall on that deferred read.
                    pool = wpool_m if g == "m" else wpool
                    strips[g] = pool.tile(
                        [P, KT, P], fp8, name=f"w_{g}", tag=f"w_{g}"
                    )
                w_ml = wpool.tile([P, KT, P], fp8, name="w_ml", tag="w_ml")
                # load order matches consumption order: i, m_hi(m),
                # m_wlo(ml), o
                w_srcs = [
                    (strips["i"], W8[d]),
                    (strips["m"], W8[DTL + d]),
                    (w_ml, WLO[d]),
                    (strips["o"], W8[2 * DTL + d]),
                ]
                if d == 0:
                    for q in range(2):
                        for si, (w_sb, src) in enumerate(w_srcs):
                            eng = nc.sync if si < 2 else nc.gpsimd
                            eng.dma_start(
                                w_sb[:, q * HK : (q + 1) * HK, :],
                                src[:, q * HK : (q + 1) * HK, :],
                            )
                    # bias rides the Pool queue after d0's strips; it isn't
                    # needed until the first epilogue.
                    nc.gpsimd.dma_start(bias_sb[:], BIAS[:])
                elif d == 1:
                    # d1's strips ride the ACT queue, issued after the A8
                    # slices: DMA_ENGINES drains FIFO-by-trigger, so putting
                    # these behind A8 keeps d0's tail k-groups (which need
                    # the last A8 slice) from being preempted.
                    for w_sb, src in w_srcs:
                        nc.scalar.dma_start(w_sb[:], src)
                else:
                    for w_sb, src in w_srcs:
                        nc.sync.dma_start(w_sb[:], src)

                def load_c(d_):
                    out = []
                    for nh in range(NH):
                        c_t = cpool.tile(
                            [P, NF], bf16, name=f"c_{nh}", tag=f"c_{nh}"
                        )
                        nc.scalar.dma_start(
                            c_t[:],
                            CT[d_ * P : (d_ + 1) * P, nh * NF : (nh + 1) * NF],
                        )
                        out.append(c_t)
                    return out

                # c tiles for tile d-1: first read (fc in d-1's m epilogue)
                # is at the end of THIS block, so loading here keeps them out
                # of the congested early DMA window.
                c_prev = load_c(d - 1) if d >= 1 else None

                # GEMM in fp8 DoubleRow mode (contract 256 per matmul).
                # Accumulation chains per (gate, nh) PSUM bank:
                #   i: A8@Wi8                       (16 matmuls)
                #   m: A8@Wm8 + A8@WmLO + ALO@Wm8   (48 matmuls)
                #   o: A8@Wo8                       (16 matmuls)
                # PSUM: i/o single-buffered (4 banks) + m double-buffered
                # (4 banks) so d0's m chain can close inside d1's block.
                psums = {}
                for g in "io":
                    for nh in range(NH):
                        psums[(g, nh)] = psum_io.tile(
                            [P, NF], f32, name=f"ps_{g}{nh}", tag=f"ps_{g}{nh}"
                        )
                for nh in range(NH):
                    psums[("m", nh)] = psum_m.tile(
                        [P, NF], f32, name=f"ps_m{nh}", tag=f"ps_m{nh}"
                    )

                def emit(g, w_sb, a, start, stop, ps=None, nk2=K2):
                    ps = ps or psums
                    for k2 in range(nk2):
                        for nh in range(NH):
                            nc.tensor.matmul(
                                ps[(g, nh)][:],
                                w_sb[:, 2 * k2 : 2 * k2 + 2, :],
                                a[:, 2 * k2 : 2 * k2 + 2, nh * NF : (nh + 1) * NF],
                                start=start and (k2 == 0),
                                stop=stop and (k2 == nk2 - 1),
                                perf_mode=DR,
                            )

                def s_i_epilogue(d_, psums_):
                    b_i = bias_sb[:, d_ : d_ + 1]
                    out = []
                    for nh in range(NH):
                        s_i = epool.tile([P, NF], f32, name="s_i", tag="s_i")
                        nc.scalar.activation(
                            s_i[:], psums_[("i", nh)][:], AF.Sigmoid,
                            bias=b_i, scale=INV,
                        )
                        out.append(s_i)
                    return out

                # Software pipeline: every d-tile's ALO pass (the third m-gate
                # accumulation term) runs at the END of block d+1, so the m
                # chain of tile d closes one block late (m PSUM banks are
                # double-buffered for this). This keeps the 4.2MB ALO tensor
                # entirely out of the congested early DMA window — any PE gap
                # there costs double, since it also resets the PE clock ramp.
                # Epilogue pieces interleave with the matmul passes so each
                # ACT/DVE stage fires as soon as its PSUM chain closes; the
                # kernel tail is just s_o -> h_new -> DMA.
                if d == 0:
                    # k-major over the A8-fed passes: PE streams right behind
                    # the A8 slice DMAs instead of stalling on the full load.
                    gate_passes = [
                        ("i", [strips["i"]]),
                        ("m", [strips["m"], w_ml]),
                        ("o", [strips["o"]]),
                    ]
                    for k2 in range(K2):
                        for g, ws in gate_passes:
                            for pi, w_sb in enumerate(ws):
                                for nh in range(NH):
                                    nc.tensor.matmul(
                                        psums[(g, nh)][:],
                                        w_sb[:, 2 * k2 : 2 * k2 + 2, :],
                                        a_sb[:, 2 * k2 : 2 * k2 + 2, nh * NF : (nh + 1) * NF],
                                        start=(pi == 0 and k2 == 0),
                                        stop=(g != "m" and k2 == K2 - 1),
                                        perf_mode=DR,
                                    )
                    s_i_tiles = s_i_epilogue(0, psums)
                    s_o_tiles = s_o_epilogue(0, psums)
                    prev = (0, psums, strips["m"], s_i_tiles, s_o_tiles)
                else:
                    if d == 1:
                        # ALO loads on the ACT queue right after d1's strips:
                        # trigger order keeps all earlier-needed transfers
                        # (a8, strips) ahead of ALO (first read ~40us in).
                        # All quarters load here — every quarter must precede
                        # its first reader (the deferred d0 ALO pass at the
                        # end of this block) in program order, or the read
                        # races the DMA.
                        for q in range(AKT // AQ):
                            nc.scalar.dma_start(
                                alo_sb[:, q * AQ : (q + 1) * AQ, :],
                                ALO[:, q * AQ : (q + 1) * AQ, :],
                            )
                    last = d == dtl - 1
                    # gate-major: each gate's PSUM bank drains (ACT) while the
                    # next gate's matmuls run.
                    emit("i", strips["i"], a_sb, start=True, stop=True)
                    s_i_tiles = s_i_epilogue(d, psums)

                    if last:
                        # Last tile: close the previous tile's m chain first,
                        # then run this tile's m chain fully in-block (ALO is
                        # long-resident) so only s_o -> h_new -> DMA remains
                        # after the final o matmul.
                        c_own = load_c(d)
                        p_d, p_psums, p_strip_m, p_s_i, p_s_o = prev
                        emit("m", p_strip_m, alo_sb, start=False, stop=True,
                             ps=p_psums, nk2=AK2)
                        p_mid = m_mid_epilogue(p_d, p_psums, p_s_i, c_prev)
                        h_epilogue(p_d, p_s_o, p_mid, nc.gpsimd)

                    emit("m", strips["m"], a_sb, start=True, stop=False)
                    emit("m", w_ml, a_sb, start=False, stop=False)
                    if last:
                        emit("m", strips["m"], alo_sb, start=False, stop=True,
                             nk2=AK2)
                        mid = m_mid_epilogue(d, psums, s_i_tiles, c_own)

                    b_o = bias_sb[:, 2 * DTL + d : 2 * DTL + d + 1]
                    s_o_tiles = []
                    for nh in range(NH):
                        # o-gate nh-major: nh0's s_o (and on the last tile its
                        # h/DMA tail) overlaps nh1's matmuls.
                        for k2 in range(K2):
                            nc.tensor.matmul(
                                psums[("o", nh)][:],
                                strips["o"][:, 2 * k2 : 2 * k2 + 2, :],
                                a_sb[:, 2 * k2 : 2 * k2 + 2,
                                     nh * NF : (nh + 1) * NF],
                                start=(k2 == 0),
                                stop=(k2 == K2 - 1),
                                perf_mode=DR,
                            )
                        s_o = epool.tile([P, NF], bf16, name="s_o", tag="s_o")
                        nc.scalar.activation(
                            s_o[:], psums[("o", nh)][:], AF.Sigmoid,
                            bias=b_o, scale=INV,
                        )
                        s_o_tiles.append(s_o)
                        if last:
                            # Outputs ride idle HWDGE queues (SWDGE's ~2.6us
                            # latency would sit on the critical tail); nh0 on
                            # ACT and nh1 on SP so the final DMA's issue
                            # pipeline isn't queued behind nh0's on one SEQ.
                            h_new = epool.tile(
                                [P, NF], bf16, name="h_new", tag="h_new"
                            )
                            nc.vector.tensor_mul(h_new[:], s_o[:], mid[nh][:])
                            nc.sync.dma_start(
                                HT[d * P : (d + 1) * P,
                                   nh * NF : (nh + 1) * NF],
                                h_new[:],
                            )

                    if not last:
                        # Close the previous tile's m chain + run its epilogue
                        # under this block's later matmuls.
                        p_d, p_psums, p_strip_m, p_s_i, p_s_o = prev
                        emit("m", p_strip_m, alo_sb, start=False, stop=True,
                             ps=p_psums, nk2=AK2)
                        p_mid = m_mid_epilogue(p_d, p_psums, p_s_i, c_prev)
                        h_epilogue(p_d, p_s_o, p_mid, nc.gpsimd)
                        prev = (d, psums, strips["m"], s_i_tiles, s_o_tiles)

    _split_multiwaits(nc)
    return nc


def _get_bass():
    if "nc" not in _CACHE:
        _CACHE["nc"] = _build_bass()
    return _CACHE["nc"]


def _prepare_in_maps(x, h, c, Wix, bix, Wmx, bmx, Wox, box, Wih, bih, Wmh, bmh, Woh, boh):
    x = np.asarray(x, dtype=np.float32)
    h = np.asarray(h, dtype=np.float32)
    c = np.asarray(c, dtype=np.float32)

    # W = [Wx ‖ Wh] with gate rows [i, m, o]: [6144, 4096], pre-scaled by SW.
    W_full = np.concatenate(
        [
            np.concatenate([np.asarray(Wix), np.asarray(Wmx), np.asarray(Wox)], axis=0),
            np.concatenate([np.asarray(Wih), np.asarray(Wmh), np.asarray(Woh)], axis=0),
        ],
        axis=1,
    ).astype(np.float32) * np.float32(SW)
    W8_f = W_full.astype(_E4)
    # m-gate residual at the same scale (captures W quantization error).
    Wm_res = (W_full[DH : 2 * DH] - W8_f[DH : 2 * DH].astype(np.float32)).astype(_E4)

    # WH[mt, p, kt, f] = W[mt*128+f, kt*128+p]
    def strip_layout(w, mt):
        return np.ascontiguousarray(
            w.reshape(mt, P, KT, P).transpose(0, 3, 2, 1)
        )

    W8_host = strip_layout(W8_f, MT)
    WLO_host = strip_layout(Wm_res, DTL)

    # A = [x ‖ h] : [8192, 4096], pre-scaled by SA -> per-core [p, kt, n]
    A = np.concatenate([x, h], axis=1) * np.float32(SA)
    A8_f = A.astype(_E4)
    A_res = (A - A8_f.astype(np.float32)).astype(_E4)

    def act_layout(a):
        return np.ascontiguousarray(
            a.reshape(N_CORES, BLOC, KT, P).transpose(0, 3, 2, 1)
        )

    A8_host = act_layout(A8_f)
    ALO_host = np.ascontiguousarray(act_layout(A_res)[:, :, :AKT, :])

    # c transposed per core: [core, 2048, 1024], bf16 (halves DMA traffic;
    # ~0.3% extra error on the forget-gate term, well within budget)
    CT_host = np.ascontiguousarray(
        c.reshape(N_CORES, BLOC, DH).transpose(0, 2, 1)
    ).astype(ml_dtypes.bfloat16)

    bias = np.concatenate(
        [
            np.asarray(bix) + np.asarray(bih),
            np.asarray(bmx) + np.asarray(bmh),
            np.asarray(box) + np.asarray(boh),
        ]
    ).astype(np.float32)
    BIAS_host = np.ascontiguousarray(bias.reshape(MT, P).T)

    return [
        {
            "W8": W8_host,
            "WLO": WLO_host,
            "A8": A8_host[core],
            "ALO": ALO_host[core],
            "CT": CT_host[core],
            "BIAS": BIAS_host,
        }
        for core in range(N_CORES)
    ]


def _postprocess(results):
    """results: per-core list of {'HT': [2048,1024], 'CNT': [2048,1024]}."""
    h_new = (
        np.stack([np.asarray(results[core]["HT"]) for core in range(N_CORES)])
        .transpose(0, 2, 1)
        .reshape(B, DH)
        .astype(np.float32)
    )
    c_new = (
        np.stack([np.asarray(results[core]["CNT"]) for core in range(N_CORES)])
        .transpose(0, 2, 1)
        .reshape(B, DH)
        .astype(np.float32)
    )
    return (h_new, c_new)


def kernel(x, h, c, Wix, bix, Wmx, bmx, Wox, box, Wih, bih, Wmh, bmh, Woh, boh):
    global LAST_RESULT
    from concourse.bass_utils import run_bass_kernel_spmd

    in_maps = _prepare_in_maps(
        x, h, c, Wix, bix, Wmx, bmx, Wox, box, Wih, bih, Wmh, bmh, Woh, boh
    )
    nc = _get_bass()
    try:
        res = run_bass_kernel_spmd(nc, in_maps, core_ids=list(range(N_CORES)))
    except ModuleNotFoundError:
        # BASS_TRACE under axon needs antenv.axon_hooks, which some
        # containers lack; fall back to an untraced run.
        os.environ["BASS_NEVER_TRACE"] = "1"
        res = run_bass_kernel_spmd(nc, in_maps, core_ids=list(range(N_CORES)))
    LAST_RESULT = res
    return _postprocess(res.results)
